# revision 3
# baseline (speedup 1.0000x reference)
"""DCMHA (DCFormer dynamically-composable multi-head attention) on 8 trn2 NeuronCores.

Sharding: 8 cores = 2 batches x 4 query-chunks of 256 tokens. Every core
holds all 16 heads for its query rows, so both cross-head projections
(which mix over the full head axis) are core-local and no collectives are
needed; each core emits a disjoint [256, 2048] slice of the output.
Per-core key/value and dynamic-weight work runs over the full 1024-token
sequence (causality handled by an input mask) so all 8 shards share one
compiled SPMD program.
"""

import numpy as np

B, T, D, N = 2, 1024, 2048, 16
HD = D // N            # 128
C, K = 4, 128
DHD = 2
NSHARD = 8
TQ = T // 4            # 256 query rows per core

_compiled = None


def _build():
    global _compiled
    if _compiled is not None:
        return _compiled
    import jax
    import jax.numpy as jnp
    try:
        jax.config.update("jax_compilation_cache_dir", "/tmp/jax_neuron_cache")
        jax.config.update("jax_persistent_cache_min_compile_time_secs", 1.0)
    except Exception:
        pass

    def shard_fn(x_full, x_q, mask, w_q, w_k, w_v, w_o, dw1r, qkw_r, dd_wr):
        q = (x_q @ w_q).reshape(TQ, N, HD).transpose(1, 0, 2) * (HD ** -0.5)
        k = (x_full @ w_k).reshape(T, N, HD).transpose(1, 0, 2)
        v = (x_full @ w_v).reshape(T, N, HD).transpose(1, 0, 2)

        def dyn(xx, L):
            dw_h = jax.nn.gelu(xx @ dw1r)
            w = jnp.einsum('tck,ckjn->tcjn', dw_h.reshape(L, C, K), qkw_r)
            w1, w2 = w[:, :, :DHD, :], w[:, :, DHD:, :]
            var = jnp.mean(w1 * w1, axis=-1, keepdims=True)
            w1 = w1 * jax.lax.rsqrt(var + 1e-6)
            dd = jnp.tanh(xx @ dd_wr).reshape(L, 4, N)
            return w1, w2, dd

        w1q, w2q, ddq = dyn(x_q, TQ)        # q-side weights (this shard's rows)
        w1k, w2k, ddk = dyn(x_full, T)      # k-side weights (all rows)

        def proj(inp, qw1, qw2, kw1, kw2, qdd, kdd):
            h_q = jnp.einsum('nts,tin->tsi', inp, qw1)
            out = inp + jnp.einsum('tsi,tin->nts', h_q, qw2)
            h_k = jnp.einsum('nts,sin->tsi', inp, kw1)
            out = out + jnp.einsum('tsi,sin->nts', h_k, kw2)
            out = out + inp * qdd.T[:, :, None]
            out = out + inp * kdd.T[:, None, :]
            return out

        logits = jnp.einsum('nth,nsh->nts', q, k)
        logits = proj(logits, w1q[:, 0], w2q[:, 0], w1k[:, 1], w2k[:, 1],
                      ddq[:, 0], ddk[:, 1])
        logits = jnp.where(mask[None, :, :], logits, jnp.finfo(jnp.float32).min)
        probs = jax.nn.softmax(logits, axis=-1)
        probs = proj(probs, w1q[:, 2], w2q[:, 2], w1k[:, 3], w2k[:, 3],
                     ddq[:, 2], ddk[:, 3])
        o = jnp.einsum('nts,nsh->nth', probs, v)
        o = o.transpose(1, 0, 2).reshape(TQ, D)
        return o @ w_o.T

    devs = jax.devices()[:NSHARD]
    f = jax.pmap(shard_fn,
                 in_axes=(0, 0, 0, None, None, None, None, None, None, None),
                 devices=devs)
    _compiled = f
    return f


def kernel(x, w_qkv, w_o, dw1, qkw, dd_w):
    x = np.asarray(x, np.float32)
    w_qkv = np.asarray(w_qkv, np.float32)
    w_o = np.asarray(w_o, np.float32)
    dw1r = np.asarray(dw1, np.float32).reshape(D, C * K)
    qkw_r = np.asarray(qkw, np.float32).reshape(C, K, 2 * DHD, N)
    dd_wr = np.asarray(dd_w, np.float32).reshape(D, 4 * N)

    w_q, w_k, w_v = w_qkv[:, :D], w_qkv[:, D:2 * D], w_qkv[:, 2 * D:]

    # shard s = b*4 + c  ->  batch b, query rows [c*TQ, (c+1)*TQ)
    x_full = np.stack([x[s // 4] for s in range(NSHARD)])            # [8, T, D]
    x_q = np.stack([x[s // 4, (s % 4) * TQ:(s % 4 + 1) * TQ]
                    for s in range(NSHARD)])                          # [8, TQ, D]
    rows = np.arange(T)
    mask = np.stack([(rows[(s % 4) * TQ:(s % 4 + 1) * TQ, None] >= rows[None, :])
                     for s in range(NSHARD)])                         # [8, TQ, T]

    f = _build()
    out_shards = np.asarray(f(x_full, x_q, mask, w_q, w_k, w_v, w_o,
                              dw1r, qkw_r, dd_wr))                    # [8, TQ, D]
    out = np.empty((B, T, D), np.float32)
    for s in range(NSHARD):
        out[s // 4, (s % 4) * TQ:(s % 4 + 1) * TQ] = out_shards[s]
    return out
